# revision 5
# baseline (speedup 1.0000x reference)
"""MaxSimilarity (cosine-sim row-max) Trainium2 kernel.

out[i] = max_j  (x1[i] . x2[j]) / max(||x1[i]|| * ||x2[j]||, 1e-8)
x1: [8192, 1024] f32, x2: [16384, 1024] f32, out: [8192] f32.

Strategy (8 NeuronCores):
- Shard x2 rows 8-way (2048 rows/core); replicate x1. Each core computes the
  row-max over its j-shard for all 8192 queries; host combines shards with
  elementwise max.
- Rows of x1 and x2 are normalized to unit length on the host, so the device
  kernel is a pure matmul + row-max. Matmul runs in float32r (TF32), which
  streams at 1 cycle/row -- a single term gives ~1e-4 relative error on this
  data, far inside the gate, so no hi/lo split. That puts the kernel at the
  PE compute roofline: 2048 matmuls x 512 cycles.
- Loop structure is jb-outer over m-panels of 16 query tiles: each of the 4
  passes over a panel reuses the panel's resident x1 tiles and needs only
  one 2 MB j-chunk of x2, so the PE starts after ~2.5 MB of DMA instead of
  waiting for the full 8 MB x2 shard (which cost 35 us of dead PE time in
  the m-outer version). x2 chunks are DMA'd in 256 KB k-slices to unblock
  the very first matmul group early.
- PSUM tiles [128 q, 512 j] are drained on DVE with a reduce-max over j into
  a per-(m,jb) column; after a panel's last pass each query tile's 4 block
  maxima are reduced and the result is written out once, contiguously, in
  [q_within_tile, m_tile] layout (the host untransposes -- a direct
  (m p)-ordered DMA scatters 8192 4-byte writes to HBM and costs ~25 us in
  write-completion latency).
"""

import numpy as np

import concourse.bacc as bacc
import concourse.mybir as mybir
import concourse.tile as tile
from concourse.bass_utils import run_bass_kernel_spmd

N1, N2, D = 8192, 16384, 1024
P = 128
NCORES = 8
JS = N2 // NCORES          # 2048 j per core
JBLK = 512                 # psum moving free dim (one bank of fp32)
JB = JS // JBLK            # 4 psum blocks per core
M_TILES = N1 // P          # 64
K_TILES = D // P           # 8
MP = 16                    # m-tiles per panel
PARTS = M_TILES // MP      # 4 panels

F32 = mybir.dt.float32
F32R = mybir.dt.float32r
ALU = mybir.AluOpType
AX = mybir.AxisListType


def tf32_round(x):
    """Round fp32 to 11 explicit mantissa bits (RNE) = float32r-representable."""
    u = x.view(np.uint32).astype(np.uint64)
    keep = np.uint64(12)
    half = np.uint64(1 << 11)
    lsb = (u >> keep) & np.uint64(1)
    u2 = (u + half - np.uint64(1) + lsb) >> keep << keep
    return u2.astype(np.uint32).view(np.float32)


def build_nc():
    nc = bacc.Bacc(trn_type="TRN2")

    x1t = nc.dram_tensor("x1t", [M_TILES, P, K_TILES, P], F32R, kind="ExternalInput")
    x2t = nc.dram_tensor("x2t", [P, K_TILES, JS], F32R, kind="ExternalInput")
    out = nc.dram_tensor("out", [P, M_TILES], F32, kind="ExternalOutput")

    with tile.TileContext(nc) as tc:
        with (
            tc.tile_pool(name="resident", bufs=1) as res,
            tc.tile_pool(name="x1pool", bufs=MP) as x1pool,
            tc.tile_pool(name="psum", bufs=8, space="PSUM") as psum,
        ):
            # resident transposed x2 shard. dma_start issue costs ~650 ns
            # each (serial on the Sync engine), so use few, big DMAs and
            # issue them in consumption order, interleaved with the first
            # panel's x1 tiles: the first matmul group is gated on DMA #1
            # (x2 j-block 0) + DMA #2 (x1 tile 0) only.
            x2t_t = res.tile([P, K_TILES, JS], F32R, tag="x2t")
            cmax = res.tile([P, M_TILES, JB], F32, tag="cmax")
            rmax = res.tile([P, M_TILES], F32, tag="rmax")

            def load_x2_chunk(jb):
                js = slice(jb * JBLK, (jb + 1) * JBLK)
                nc.sync.dma_start(out=x2t_t[:, :, js], in_=x2t[:, :, js])

            def load_x1(m):
                a = x1pool.tile([P, K_TILES, P], F32R, tag="x1")
                nc.sync.dma_start(out=a[:], in_=x1t[m])
                return a

            for part in range(PARTS):
                tiles = []
                if part == 0:
                    for jb in range(JB):
                        load_x2_chunk(jb)
                        tiles += [load_x1(m) for m in range(jb * 4, jb * 4 + 4)]
                else:
                    tiles = [load_x1(part * MP + mi) for mi in range(MP)]

                if part == 0:
                    # j-block-outer passes: pass jb only needs x2 chunk jb,
                    # so the PE starts ~2.5 MB into the load, not 8 MB
                    for jb in range(JB):
                        js = slice(jb * JBLK, (jb + 1) * JBLK)
                        for mi in range(MP):
                            m = part * MP + mi
                            ps = psum.tile([P, JBLK], F32, tag="ps")
                            for k in range(K_TILES):
                                nc.tensor.matmul(
                                    ps[:], tiles[mi][:, k, :], x2t_t[:, k, js],
                                    start=(k == 0), stop=(k == K_TILES - 1),
                                )
                            nc.vector.tensor_reduce(
                                cmax[:, m, jb : jb + 1], ps[:], axis=AX.X, op=ALU.max
                            )
                            if jb == JB - 1:
                                nc.vector.tensor_reduce(
                                    rmax[:, m : m + 1], cmax[:, m, :], axis=AX.X, op=ALU.max
                                )
                else:
                    # x2 fully resident now: k-outer order reuses each
                    # stationary for 4 j-block matmuls (amortizes LDWEIGHTS)
                    for mi in range(MP):
                        m = part * MP + mi
                        pss = [psum.tile([P, JBLK], F32, tag="ps", name="ps") for _ in range(JB)]
                        for k in range(K_TILES):
                            for jb in range(JB):
                                js = slice(jb * JBLK, (jb + 1) * JBLK)
                                nc.tensor.matmul(
                                    pss[jb][:], tiles[mi][:, k, :], x2t_t[:, k, js],
                                    start=(k == 0), stop=(k == K_TILES - 1),
                                )
                        for jb in range(JB):
                            nc.vector.tensor_reduce(
                                cmax[:, m, jb : jb + 1], pss[jb][:], axis=AX.X, op=ALU.max
                            )
                        nc.vector.tensor_reduce(
                            rmax[:, m : m + 1], cmax[:, m, :], axis=AX.X, op=ALU.max
                        )

            nc.sync.dma_start(out=out[:], in_=rmax[:])

    nc.finalize()
    return nc


_cache = {}


def _get_nc():
    if "nc" not in _cache:
        _cache["nc"] = build_nc()
    return _cache["nc"]


def _prep_inputs(x1, x2):
    """Host-side prep: row-normalize, TF32-round, transpose + tile + shard."""
    x1 = np.ascontiguousarray(x1, dtype=np.float32)
    x2 = np.ascontiguousarray(x2, dtype=np.float32)
    eps = np.float32(1e-8)
    n1 = np.maximum(np.sqrt(np.einsum("ij,ij->i", x1, x1)), eps)
    n2 = np.maximum(np.sqrt(np.einsum("ij,ij->i", x2, x2)), eps)
    x1 = tf32_round(x1 / n1[:, None])
    x2 = tf32_round(x2 / n2[:, None])

    # [N1, D] -> [m, dp, k, q] with x1t[m, dp, k, q] = x1[m*128+q, k*128+dp]
    x1t = np.ascontiguousarray(
        x1.reshape(M_TILES, P, K_TILES, P).transpose(0, 3, 2, 1)
    )

    in_maps = []
    for c in range(NCORES):
        sl = slice(c * JS, (c + 1) * JS)
        # [JS, D] -> [dp, k, j] with x2t[dp, k, j] = x2[sl][j, k*128+dp]
        x2t = np.ascontiguousarray(
            x2[sl].T.reshape(K_TILES, P, JS).transpose(1, 0, 2)
        )
        in_maps.append({"x1t": x1t, "x2t": x2t})
    return in_maps


def run(x1, x2, trace=False):
    nc = _get_nc()
    in_maps = _prep_inputs(x1, x2)
    res = run_bass_kernel_spmd(nc, in_maps, core_ids=list(range(NCORES)), trace=trace)
    # device output is [q_within_tile, m_tile]; out[m*128+q] = arr[q, m]
    parts = [np.asarray(res.results[c]["out"]).reshape(P, M_TILES) for c in range(NCORES)]
    out = np.maximum.reduce(parts).T.ravel().astype(np.float32)
    return np.ascontiguousarray(out), res


def kernel(x1, x2):
    out, _ = run(np.asarray(x1), np.asarray(x2), trace=False)
    return out


# revision 6
# speedup vs baseline: 1.3556x; 1.3556x over previous
"""MaxSimilarity (cosine-sim row-max) Trainium2 kernel.

out[i] = max_j  (x1[i] . x2[j]) / max(||x1[i]|| * ||x2[j]||, 1e-8)
x1: [8192, 1024] f32, x2: [16384, 1024] f32, out: [8192] f32.

Strategy (8 NeuronCores):
- Shard x2 rows 8-way (2048 rows/core); replicate x1. Each core computes the
  row-max over its j-shard for all 8192 queries; host combines shards with
  elementwise max.
- Rows of x1 and x2 are normalized to unit length on the host, so the device
  kernel is a pure matmul + row-max. Matmul runs in float32r (TF32), which
  streams at 1 cycle/row -- a single term gives ~1e-4 relative error on this
  data, far inside the gate, so no hi/lo split. That puts the kernel at the
  PE compute roofline: 2048 matmuls x 512 cycles.
- Loop structure is jb-outer over m-panels of 16 query tiles: each of the 4
  passes over a panel reuses the panel's resident x1 tiles and needs only
  one 2 MB j-chunk of x2, so the PE starts after ~2.5 MB of DMA instead of
  waiting for the full 8 MB x2 shard (which cost 35 us of dead PE time in
  the m-outer version). x2 chunks are DMA'd in 256 KB k-slices to unblock
  the very first matmul group early.
- PSUM tiles [128 q, 512 j] are drained on DVE with a reduce-max over j into
  a per-(m,jb) column; after a panel's last pass each query tile's 4 block
  maxima are reduced and the result is written out once, contiguously, in
  [q_within_tile, m_tile] layout (the host untransposes -- a direct
  (m p)-ordered DMA scatters 8192 4-byte writes to HBM and costs ~25 us in
  write-completion latency).
"""

import numpy as np

import concourse.bacc as bacc
import concourse.mybir as mybir
import concourse.tile as tile
from concourse.bass_utils import run_bass_kernel_spmd

N1, N2, D = 8192, 16384, 1024
P = 128
NCORES = 8
JS = N2 // NCORES          # 2048 j per core
JBLK = 512                 # psum moving free dim (one bank of fp32)
JB = JS // JBLK            # 4 psum blocks per core
M_TILES = N1 // P          # 64
K_TILES = D // P           # 8
MP = 16                    # m-tiles per panel
PARTS = M_TILES // MP      # 4 panels

F32 = mybir.dt.float32
F32R = mybir.dt.float32r
ALU = mybir.AluOpType
AX = mybir.AxisListType


def tf32_round(x):
    """Round fp32 to 11 explicit mantissa bits (RNE) = float32r-representable."""
    u = x.view(np.uint32).astype(np.uint64)
    keep = np.uint64(12)
    half = np.uint64(1 << 11)
    lsb = (u >> keep) & np.uint64(1)
    u2 = (u + half - np.uint64(1) + lsb) >> keep << keep
    return u2.astype(np.uint32).view(np.float32)


def build_nc():
    nc = bacc.Bacc(trn_type="TRN2")

    x1t = nc.dram_tensor("x1t", [M_TILES, P, K_TILES, P], F32R, kind="ExternalInput")
    x2t = nc.dram_tensor("x2t", [P, K_TILES, JS], F32R, kind="ExternalInput")
    out = nc.dram_tensor("out", [P, M_TILES], F32, kind="ExternalOutput")

    with tile.TileContext(nc) as tc:
        with (
            tc.tile_pool(name="resident", bufs=1) as res,
            tc.tile_pool(name="x1pool", bufs=MP) as x1pool,
            tc.tile_pool(name="psum", bufs=8, space="PSUM") as psum,
        ):
            # resident transposed x2 shard. dma_start issue costs ~650 ns
            # each (serial on the Sync engine), so use few, big DMAs and
            # issue them in consumption order, interleaved with the first
            # panel's x1 tiles: the first matmul group is gated on DMA #1
            # (x2 j-block 0) + DMA #2 (x1 tile 0) only.
            x2t_t = res.tile([P, K_TILES, JS], F32R, tag="x2t")
            cmax = res.tile([P, M_TILES, JB], F32, tag="cmax")
            rmax = res.tile([P, M_TILES], F32, tag="rmax")

            def load_x2_chunk(jb):
                js = slice(jb * JBLK, (jb + 1) * JBLK)
                nc.sync.dma_start(out=x2t_t[:, :, js], in_=x2t[:, :, js])

            def load_x1(m):
                a = x1pool.tile([P, K_TILES, P], F32R, tag="x1")
                nc.sync.dma_start(out=a[:], in_=x1t[m])
                return a

            # (m_start, m_count, order). The two leading 8-tile subparts run
            # j-block-outer so the PE starts on x2 chunk 0 + one x1 tile and
            # never outruns the DMA (x1 demand during the first pass is
            # ~235 GB/s; keeping it to 8 tiles leaves headroom for the
            # remaining x2 chunks). Middle parts run k-outer (stationary
            # reused for 4 j-blocks). The last part runs j-block-outer so
            # the final drains interleave with matmuls instead of bunching
            # after the last one.
            parts = [(0, 8, "jb"), (8, 8, "jb"), (16, 16, "k"),
                     (32, 16, "k"), (48, 16, "jb")]

            def jb_outer(tiles, m0, cnt):
                for jb in range(JB):
                    js = slice(jb * JBLK, (jb + 1) * JBLK)
                    for mi in range(cnt):
                        m = m0 + mi
                        ps = psum.tile([P, JBLK], F32, tag="ps")
                        for k in range(K_TILES):
                            nc.tensor.matmul(
                                ps[:], tiles[mi][:, k, :], x2t_t[:, k, js],
                                start=(k == 0), stop=(k == K_TILES - 1),
                            )
                        nc.vector.tensor_reduce(
                            cmax[:, m, jb : jb + 1], ps[:], axis=AX.X, op=ALU.max
                        )
                        if jb == JB - 1:
                            nc.vector.tensor_reduce(
                                rmax[:, m : m + 1], cmax[:, m, :], axis=AX.X, op=ALU.max
                            )

            def k_outer(tiles, m0, cnt):
                for mi in range(cnt):
                    m = m0 + mi
                    pss = [psum.tile([P, JBLK], F32, tag="ps", name="ps") for _ in range(JB)]
                    for k in range(K_TILES):
                        for jb in range(JB):
                            js = slice(jb * JBLK, (jb + 1) * JBLK)
                            nc.tensor.matmul(
                                pss[jb][:], tiles[mi][:, k, :], x2t_t[:, k, js],
                                start=(k == 0), stop=(k == K_TILES - 1),
                            )
                    for jb in range(JB):
                        nc.vector.tensor_reduce(
                            cmax[:, m, jb : jb + 1], pss[jb][:], axis=AX.X, op=ALU.max
                        )
                    nc.vector.tensor_reduce(
                        rmax[:, m : m + 1], cmax[:, m, :], axis=AX.X, op=ALU.max
                    )

            for pi, (m0, cnt, order) in enumerate(parts):
                if pi == 0:
                    load_x2_chunk(0)
                    tiles = [load_x1(m0 + mi) for mi in range(cnt)]
                    for jb in range(1, JB):
                        load_x2_chunk(jb)
                else:
                    tiles = [load_x1(m0 + mi) for mi in range(cnt)]
                (jb_outer if order == "jb" else k_outer)(tiles, m0, cnt)
                # flush this part's finished output columns; keeps the final
                # HBM write (and its completion wait) small
                nc.sync.dma_start(
                    out=out[:, m0 : m0 + cnt], in_=rmax[:, m0 : m0 + cnt]
                )

    nc.finalize()
    return nc


_cache = {}


def _get_nc():
    if "nc" not in _cache:
        _cache["nc"] = build_nc()
    return _cache["nc"]


def _prep_inputs(x1, x2):
    """Host-side prep: row-normalize, TF32-round, transpose + tile + shard."""
    x1 = np.ascontiguousarray(x1, dtype=np.float32)
    x2 = np.ascontiguousarray(x2, dtype=np.float32)
    eps = np.float32(1e-8)
    n1 = np.maximum(np.sqrt(np.einsum("ij,ij->i", x1, x1)), eps)
    n2 = np.maximum(np.sqrt(np.einsum("ij,ij->i", x2, x2)), eps)
    x1 = tf32_round(x1 / n1[:, None])
    x2 = tf32_round(x2 / n2[:, None])

    # [N1, D] -> [m, dp, k, q] with x1t[m, dp, k, q] = x1[m*128+q, k*128+dp]
    x1t = np.ascontiguousarray(
        x1.reshape(M_TILES, P, K_TILES, P).transpose(0, 3, 2, 1)
    )

    in_maps = []
    for c in range(NCORES):
        sl = slice(c * JS, (c + 1) * JS)
        # [JS, D] -> [dp, k, j] with x2t[dp, k, j] = x2[sl][j, k*128+dp]
        x2t = np.ascontiguousarray(
            x2[sl].T.reshape(K_TILES, P, JS).transpose(1, 0, 2)
        )
        in_maps.append({"x1t": x1t, "x2t": x2t})
    return in_maps


def run(x1, x2, trace=False):
    nc = _get_nc()
    in_maps = _prep_inputs(x1, x2)
    res = run_bass_kernel_spmd(nc, in_maps, core_ids=list(range(NCORES)), trace=trace)
    # device output is [q_within_tile, m_tile]; out[m*128+q] = arr[q, m]
    parts = [np.asarray(res.results[c]["out"]).reshape(P, M_TILES) for c in range(NCORES)]
    out = np.maximum.reduce(parts).T.ravel().astype(np.float32)
    return np.ascontiguousarray(out), res


def kernel(x1, x2):
    out, _ = run(np.asarray(x1), np.asarray(x2), trace=False)
    return out


# revision 8
# speedup vs baseline: 1.3971x; 1.0306x over previous
"""MaxSimilarity (cosine-sim row-max) Trainium2 kernel.

out[i] = max_j  (x1[i] . x2[j]) / max(||x1[i]|| * ||x2[j]||, 1e-8)
x1: [8192, 1024] f32, x2: [16384, 1024] f32, out: [8192] f32.

Strategy (8 NeuronCores):
- Shard x2 rows 8-way (2048 rows/core); replicate x1. Each core computes the
  row-max over its j-shard for all 8192 queries; host combines shards with
  elementwise max.
- Rows of x1 and x2 are normalized to unit length on the host, so the device
  kernel is a pure matmul + row-max. Matmul runs in float32r (TF32), which
  streams at 1 cycle/row -- a single term gives ~1e-4 relative error on this
  data, far inside the gate, so no hi/lo split. That puts the kernel at the
  PE compute roofline: 2048 matmuls x 512 cycles.
- Loop structure is jb-outer over m-panels of 16 query tiles: each of the 4
  passes over a panel reuses the panel's resident x1 tiles and needs only
  one 2 MB j-chunk of x2, so the PE starts after ~2.5 MB of DMA instead of
  waiting for the full 8 MB x2 shard (which cost 35 us of dead PE time in
  the m-outer version). x2 chunks are DMA'd in 256 KB k-slices to unblock
  the very first matmul group early.
- PSUM tiles [128 q, 512 j] are drained on DVE with a reduce-max over j into
  a per-(m,jb) column; after a panel's last pass each query tile's 4 block
  maxima are reduced and the result is written out once, contiguously, in
  [q_within_tile, m_tile] layout (the host untransposes -- a direct
  (m p)-ordered DMA scatters 8192 4-byte writes to HBM and costs ~25 us in
  write-completion latency).
"""

import numpy as np

import concourse.bacc as bacc
import concourse.mybir as mybir
import concourse.tile as tile
from concourse.bass_utils import run_bass_kernel_spmd

N1, N2, D = 8192, 16384, 1024
P = 128
NCORES = 8
JS = N2 // NCORES          # 2048 j per core
JBLK = 512                 # psum moving free dim (one bank of fp32)
JB = JS // JBLK            # 4 psum blocks per core
M_TILES = N1 // P          # 64
K_TILES = D // P           # 8
MP = 32                    # m-tiles per panel (16 MB of x1 resident)
PARTS = M_TILES // MP      # 2 panels

F32 = mybir.dt.float32
F32R = mybir.dt.float32r
ALU = mybir.AluOpType
AX = mybir.AxisListType


def tf32_round(x):
    """Round fp32 to 11 explicit mantissa bits (RNE) = float32r-representable."""
    u = x.view(np.uint32).astype(np.uint64)
    keep = np.uint64(12)
    half = np.uint64(1 << 11)
    lsb = (u >> keep) & np.uint64(1)
    u2 = (u + half - np.uint64(1) + lsb) >> keep << keep
    return u2.astype(np.uint32).view(np.float32)


def build_nc():
    nc = bacc.Bacc(trn_type="TRN2")

    x1t = nc.dram_tensor("x1t", [M_TILES, P, K_TILES, P], F32R, kind="ExternalInput")
    x2t = nc.dram_tensor("x2t", [P, K_TILES, JS], F32R, kind="ExternalInput")
    out = nc.dram_tensor("out", [P, M_TILES], F32, kind="ExternalOutput")

    with tile.TileContext(nc) as tc:
        with (
            tc.tile_pool(name="resident", bufs=1) as res,
            tc.tile_pool(name="x1pool", bufs=MP) as x1pool,
            tc.tile_pool(name="psum", bufs=8, space="PSUM") as psum,
        ):
            # resident transposed x2 shard. dma_start issue costs ~650 ns
            # each (serial on the Sync engine), so use few, big DMAs and
            # issue them in consumption order, interleaved with the first
            # panel's x1 tiles: the first matmul group is gated on DMA #1
            # (x2 j-block 0) + DMA #2 (x1 tile 0) only.
            x2t_t = res.tile([P, K_TILES, JS], F32R, tag="x2t")
            cmax = res.tile([P, M_TILES, JB], F32, tag="cmax")
            rmax = res.tile([P, M_TILES], F32, tag="rmax")

            def load_x2_chunk(jb):
                js = slice(jb * JBLK, (jb + 1) * JBLK)
                nc.sync.dma_start(out=x2t_t[:, :, js], in_=x2t[:, :, js])

            def load_x1(m):
                a = x1pool.tile([P, K_TILES, P], F32R, tag="x1")
                nc.sync.dma_start(out=a[:], in_=x1t[m])
                return a

            # (m_start, m_count, order). Two 32-tile panels, both
            # j-block-outer: the first pass over 32 query tiles runs ~68 us
            # off x2 chunk 0 alone, which is ample time for chunks 1-3 to
            # land (an 8-tile first pass was measured to outrun the DMA and
            # trip a HAM re-throttle). j-block-outer also interleaves the
            # PSUM drains with matmuls, so nothing bunches after the last MM.
            parts = [(0, 32, "jb"), (32, 32, "jb")]

            def jb_outer(tiles, m0, cnt):
                for jb in range(JB):
                    js = slice(jb * JBLK, (jb + 1) * JBLK)
                    for mi in range(cnt):
                        m = m0 + mi
                        ps = psum.tile([P, JBLK], F32, tag="ps")
                        for k in range(K_TILES):
                            nc.tensor.matmul(
                                ps[:], tiles[mi][:, k, :], x2t_t[:, k, js],
                                start=(k == 0), stop=(k == K_TILES - 1),
                            )
                        nc.vector.tensor_reduce(
                            cmax[:, m, jb : jb + 1], ps[:], axis=AX.X, op=ALU.max
                        )
                        if jb == JB - 1:
                            nc.vector.tensor_reduce(
                                rmax[:, m : m + 1], cmax[:, m, :], axis=AX.X, op=ALU.max
                            )

            def k_outer(tiles, m0, cnt):
                for mi in range(cnt):
                    m = m0 + mi
                    pss = [psum.tile([P, JBLK], F32, tag="ps", name="ps") for _ in range(JB)]
                    for k in range(K_TILES):
                        for jb in range(JB):
                            js = slice(jb * JBLK, (jb + 1) * JBLK)
                            nc.tensor.matmul(
                                pss[jb][:], tiles[mi][:, k, :], x2t_t[:, k, js],
                                start=(k == 0), stop=(k == K_TILES - 1),
                            )
                    for jb in range(JB):
                        nc.vector.tensor_reduce(
                            cmax[:, m, jb : jb + 1], pss[jb][:], axis=AX.X, op=ALU.max
                        )
                    nc.vector.tensor_reduce(
                        rmax[:, m : m + 1], cmax[:, m, :], axis=AX.X, op=ALU.max
                    )

            for pi, (m0, cnt, order) in enumerate(parts):
                if pi == 0:
                    load_x2_chunk(0)
                    tiles = [load_x1(m0 + mi) for mi in range(cnt)]
                    for jb in range(1, JB):
                        load_x2_chunk(jb)
                else:
                    tiles = [load_x1(m0 + mi) for mi in range(cnt)]
                (jb_outer if order == "jb" else k_outer)(tiles, m0, cnt)
                # flush this part's finished output columns; keeps the final
                # HBM write (and its completion wait) small
                nc.sync.dma_start(
                    out=out[:, m0 : m0 + cnt], in_=rmax[:, m0 : m0 + cnt]
                )

    nc.finalize()
    return nc


_cache = {}


def _get_nc():
    if "nc" not in _cache:
        _cache["nc"] = build_nc()
    return _cache["nc"]


def _prep_inputs(x1, x2):
    """Host-side prep: row-normalize, TF32-round, transpose + tile + shard."""
    x1 = np.ascontiguousarray(x1, dtype=np.float32)
    x2 = np.ascontiguousarray(x2, dtype=np.float32)
    eps = np.float32(1e-8)
    n1 = np.maximum(np.sqrt(np.einsum("ij,ij->i", x1, x1)), eps)
    n2 = np.maximum(np.sqrt(np.einsum("ij,ij->i", x2, x2)), eps)
    x1 = tf32_round(x1 / n1[:, None])
    x2 = tf32_round(x2 / n2[:, None])

    # [N1, D] -> [m, dp, k, q] with x1t[m, dp, k, q] = x1[m*128+q, k*128+dp]
    x1t = np.ascontiguousarray(
        x1.reshape(M_TILES, P, K_TILES, P).transpose(0, 3, 2, 1)
    )

    in_maps = []
    for c in range(NCORES):
        sl = slice(c * JS, (c + 1) * JS)
        # [JS, D] -> [dp, k, j] with x2t[dp, k, j] = x2[sl][j, k*128+dp]
        x2t = np.ascontiguousarray(
            x2[sl].T.reshape(K_TILES, P, JS).transpose(1, 0, 2)
        )
        in_maps.append({"x1t": x1t, "x2t": x2t})
    return in_maps


def run(x1, x2, trace=False):
    nc = _get_nc()
    in_maps = _prep_inputs(x1, x2)
    res = run_bass_kernel_spmd(nc, in_maps, core_ids=list(range(NCORES)), trace=trace)
    # device output is [q_within_tile, m_tile]; out[m*128+q] = arr[q, m]
    parts = [np.asarray(res.results[c]["out"]).reshape(P, M_TILES) for c in range(NCORES)]
    out = np.maximum.reduce(parts).T.ravel().astype(np.float32)
    return np.ascontiguousarray(out), res


def kernel(x1, x2):
    out, _ = run(np.asarray(x1), np.asarray(x2), trace=False)
    return out


# revision 9
# speedup vs baseline: 1.3982x; 1.0007x over previous
"""MaxSimilarity (cosine-sim row-max) Trainium2 kernel.

out[i] = max_j  (x1[i] . x2[j]) / max(||x1[i]|| * ||x2[j]||, 1e-8)
x1: [8192, 1024] f32, x2: [16384, 1024] f32, out: [8192] f32.

Strategy (8 NeuronCores):
- Shard x2 rows 8-way (2048 rows/core); replicate x1. Each core computes the
  row-max over its j-shard for all 8192 queries; host combines shards with
  elementwise max.
- Rows of x1 and x2 are normalized to unit length on the host, so the device
  kernel is a pure matmul + row-max. Matmul runs in float32r (TF32), which
  streams at 1 cycle/row -- a single term gives ~1e-4 relative error on this
  data, far inside the gate, so no hi/lo split. That puts the kernel at the
  PE compute roofline: 2048 matmuls x 512 cycles.
- Loop structure is j-block-outer over two resident panels of 32 query
  tiles: each pass over a panel needs only one 2 MB j-chunk of x2, so the
  PE starts after ~2.5 MB of DMA instead of waiting for the full 8 MB x2
  shard (which cost 35 us of dead PE time m-outer), and the ~68 us first
  pass gives the remaining chunks ample time to land (shorter first passes
  were measured to outrun the DMA and trip a HAM re-throttle).
- PSUM tiles [128 q, 512 j] are drained on DVE with a reduce-max over j into
  a per-(m,jb) column; after a panel's last pass each query tile's 4 block
  maxima are reduced and the result is written out once, contiguously, in
  [q_within_tile, m_tile] layout (the host untransposes -- a direct
  (m p)-ordered DMA scatters 8192 4-byte writes to HBM and costs ~25 us in
  write-completion latency).
"""

import numpy as np

import concourse.bacc as bacc
import concourse.mybir as mybir
import concourse.tile as tile
from concourse.bass_utils import run_bass_kernel_spmd

N1, N2, D = 8192, 16384, 1024
P = 128
NCORES = 8
JS = N2 // NCORES          # 2048 j per core
JBLK = 512                 # psum moving free dim (one bank of fp32)
JB = JS // JBLK            # 4 psum blocks per core
M_TILES = N1 // P          # 64
K_TILES = D // P           # 8
MP = 32                    # m-tiles per panel (16 MB of x1 resident)
PARTS = M_TILES // MP      # 2 panels

F32 = mybir.dt.float32
F32R = mybir.dt.float32r
ALU = mybir.AluOpType
AX = mybir.AxisListType


def tf32_round(x):
    """Round fp32 to 11 explicit mantissa bits (RNE) = float32r-representable."""
    u = x.view(np.uint32).astype(np.uint64)
    keep = np.uint64(12)
    half = np.uint64(1 << 11)
    lsb = (u >> keep) & np.uint64(1)
    u2 = (u + half - np.uint64(1) + lsb) >> keep << keep
    return u2.astype(np.uint32).view(np.float32)


def build_nc():
    nc = bacc.Bacc(trn_type="TRN2")

    x1t = nc.dram_tensor("x1t", [M_TILES, P, K_TILES, P], F32R, kind="ExternalInput")
    x2t = nc.dram_tensor("x2t", [P, K_TILES, JS], F32R, kind="ExternalInput")
    out = nc.dram_tensor("out", [P, M_TILES], F32, kind="ExternalOutput")

    with tile.TileContext(nc) as tc:
        with (
            tc.tile_pool(name="resident", bufs=1) as res,
            tc.tile_pool(name="x1pool", bufs=MP) as x1pool,
            tc.tile_pool(name="psum", bufs=8, space="PSUM") as psum,
        ):
            # resident transposed x2 shard. dma_start issue costs ~650 ns
            # each (serial on the Sync engine), so use few, big DMAs and
            # issue them in consumption order, interleaved with the first
            # panel's x1 tiles: the first matmul group is gated on DMA #1
            # (x2 j-block 0) + DMA #2 (x1 tile 0) only.
            x2t_t = res.tile([P, K_TILES, JS], F32R, tag="x2t")
            cmax = res.tile([P, M_TILES, JB], F32, tag="cmax")
            rmax = res.tile([P, M_TILES], F32, tag="rmax")

            def load_x2_chunk(jb):
                js = slice(jb * JBLK, (jb + 1) * JBLK)
                nc.sync.dma_start(out=x2t_t[:, :, js], in_=x2t[:, :, js])

            def load_x1(m):
                a = x1pool.tile([P, K_TILES, P], F32R, tag="x1")
                nc.sync.dma_start(out=a[:], in_=x1t[m])
                return a

            # (m_start, m_count, order). Two 32-tile panels, both
            # j-block-outer: the first pass over 32 query tiles runs ~68 us
            # off x2 chunk 0 alone, which is ample time for chunks 1-3 to
            # land (an 8-tile first pass was measured to outrun the DMA and
            # trip a HAM re-throttle). j-block-outer also interleaves the
            # PSUM drains with matmuls, so nothing bunches after the last MM.
            parts = [(0, 32, "jb"), (32, 32, "jb")]

            def jb_outer(tiles, m0, cnt):
                for jb in range(JB):
                    js = slice(jb * JBLK, (jb + 1) * JBLK)
                    for mi in range(cnt):
                        m = m0 + mi
                        ps = psum.tile([P, JBLK], F32, tag="ps")
                        for k in range(K_TILES):
                            nc.tensor.matmul(
                                ps[:], tiles[mi][:, k, :], x2t_t[:, k, js],
                                start=(k == 0), stop=(k == K_TILES - 1),
                            )
                        nc.vector.tensor_reduce(
                            cmax[:, m, jb : jb + 1], ps[:], axis=AX.X, op=ALU.max
                        )
                        if jb == JB - 1:
                            nc.vector.tensor_reduce(
                                rmax[:, m : m + 1], cmax[:, m, :], axis=AX.X, op=ALU.max
                            )

            def k_outer(tiles, m0, cnt):
                for mi in range(cnt):
                    m = m0 + mi
                    pss = [psum.tile([P, JBLK], F32, tag="ps", name="ps") for _ in range(JB)]
                    for k in range(K_TILES):
                        for jb in range(JB):
                            js = slice(jb * JBLK, (jb + 1) * JBLK)
                            nc.tensor.matmul(
                                pss[jb][:], tiles[mi][:, k, :], x2t_t[:, k, js],
                                start=(k == 0), stop=(k == K_TILES - 1),
                            )
                    for jb in range(JB):
                        nc.vector.tensor_reduce(
                            cmax[:, m, jb : jb + 1], pss[jb][:], axis=AX.X, op=ALU.max
                        )
                    nc.vector.tensor_reduce(
                        rmax[:, m : m + 1], cmax[:, m, :], axis=AX.X, op=ALU.max
                    )

            for pi, (m0, cnt, order) in enumerate(parts):
                if pi == 0:
                    load_x2_chunk(0)
                    tiles = [load_x1(m0 + mi) for mi in range(cnt)]
                    for jb in range(1, JB):
                        load_x2_chunk(jb)
                else:
                    tiles = [load_x1(m0 + mi) for mi in range(cnt)]
                (jb_outer if order == "jb" else k_outer)(tiles, m0, cnt)
                # flush this part's finished output columns; keeps the final
                # HBM write (and its completion wait) small
                nc.sync.dma_start(
                    out=out[:, m0 : m0 + cnt], in_=rmax[:, m0 : m0 + cnt]
                )

    nc.finalize()
    return nc


_cache = {}


def _get_nc():
    if "nc" not in _cache:
        _cache["nc"] = build_nc()
    return _cache["nc"]


def _prep_inputs(x1, x2):
    """Host-side prep: row-normalize, TF32-round, transpose + tile + shard."""
    x1 = np.ascontiguousarray(x1, dtype=np.float32)
    x2 = np.ascontiguousarray(x2, dtype=np.float32)
    eps = np.float32(1e-8)
    n1 = np.maximum(np.sqrt(np.einsum("ij,ij->i", x1, x1)), eps)
    n2 = np.maximum(np.sqrt(np.einsum("ij,ij->i", x2, x2)), eps)
    x1 = tf32_round(x1 / n1[:, None])
    x2 = tf32_round(x2 / n2[:, None])

    # [N1, D] -> [m, dp, k, q] with x1t[m, dp, k, q] = x1[m*128+q, k*128+dp]
    x1t = np.ascontiguousarray(
        x1.reshape(M_TILES, P, K_TILES, P).transpose(0, 3, 2, 1)
    )

    in_maps = []
    for c in range(NCORES):
        sl = slice(c * JS, (c + 1) * JS)
        # [JS, D] -> [dp, k, j] with x2t[dp, k, j] = x2[sl][j, k*128+dp]
        x2t = np.ascontiguousarray(
            x2[sl].T.reshape(K_TILES, P, JS).transpose(1, 0, 2)
        )
        in_maps.append({"x1t": x1t, "x2t": x2t})
    return in_maps


def run(x1, x2, trace=False):
    nc = _get_nc()
    in_maps = _prep_inputs(x1, x2)
    res = run_bass_kernel_spmd(nc, in_maps, core_ids=list(range(NCORES)), trace=trace)
    # device output is [q_within_tile, m_tile]; out[m*128+q] = arr[q, m]
    parts = [np.asarray(res.results[c]["out"]).reshape(P, M_TILES) for c in range(NCORES)]
    out = np.maximum.reduce(parts).T.ravel().astype(np.float32)
    return np.ascontiguousarray(out), res


def kernel(x1, x2):
    out, _ = run(np.asarray(x1), np.asarray(x2), trace=False)
    return out


# revision 15
# speedup vs baseline: 1.4902x; 1.0658x over previous
"""MaxSimilarity (cosine-sim row-max) Trainium2 kernel.

out[i] = max_j  (x1[i] . x2[j]) / max(||x1[i]|| * ||x2[j]||, 1e-8)
x1: [8192, 1024] f32, x2: [16384, 1024] f32, out: [8192] f32.

Strategy (8 NeuronCores):
- Shard x2 rows 8-way (2048 rows/core); replicate x1. Each core computes the
  row-max over its j-shard for all 8192 queries; host combines shards with
  elementwise max.
- Rows of x1 and x2 are normalized to unit length on the host, so the device
  kernel is a pure matmul + row-max. Matmul runs in bf16: a single term
  gives ~1.3e-3 relative error on this data, far inside the 2e-2 gate.
  bf16 streams at 1 cycle/row like float32r, but (unlike float32r, whose
  4-byte weight path forces a full weight reload with every matmul) it gets
  Fast Weight Load, which shaves the per-matmul LDWEIGHTS overhead; it also
  halves the HBM traffic. 2048 matmuls x 512 cycles per core.
- Loop structure is j-block-outer over two resident panels of 32 query
  tiles: each pass over a panel needs only one 2 MB j-chunk of x2, so the
  PE starts after ~2.5 MB of DMA instead of waiting for the full 8 MB x2
  shard (which cost 35 us of dead PE time m-outer), and the ~68 us first
  pass gives the remaining chunks ample time to land (shorter first passes
  were measured to outrun the DMA and trip a HAM re-throttle).
- PSUM tiles [128 q, 512 j] are drained on DVE with a reduce-max over j into
  a per-(m,jb) column; after a panel's last pass each query tile's 4 block
  maxima are reduced and the result is written out once, contiguously, in
  [q_within_tile, m_tile] layout (the host untransposes -- a direct
  (m p)-ordered DMA scatters 8192 4-byte writes to HBM and costs ~25 us in
  write-completion latency).
"""

import ml_dtypes
import numpy as np

import concourse.bacc as bacc
import concourse.mybir as mybir
import concourse.tile as tile
from concourse.bass_utils import run_bass_kernel_spmd

N1, N2, D = 8192, 16384, 1024
P = 128
NCORES = 8
JS = N2 // NCORES          # 2048 j per core
JBLK = 512                 # psum moving free dim (one bank of fp32)
JB = JS // JBLK            # 4 psum blocks per core
M_TILES = N1 // P          # 64
K_TILES = D // P           # 8
MP = 32                    # m-tiles per panel (16 MB of x1 resident)
PARTS = M_TILES // MP      # 2 panels

F32 = mybir.dt.float32
BF16 = mybir.dt.bfloat16
ALU = mybir.AluOpType
AX = mybir.AxisListType


def build_nc():
    nc = bacc.Bacc(trn_type="TRN2")

    x1t = nc.dram_tensor("x1t", [M_TILES, P, K_TILES, P], BF16, kind="ExternalInput")
    x2t = nc.dram_tensor("x2t", [P, K_TILES, JS], BF16, kind="ExternalInput")
    out = nc.dram_tensor("out", [P, M_TILES], F32, kind="ExternalOutput")

    with tile.TileContext(nc) as tc:
        with (
            tc.tile_pool(name="resident", bufs=1) as res,
            tc.tile_pool(name="x1pool", bufs=MP) as x1pool,
            tc.tile_pool(name="psum", bufs=8, space="PSUM") as psum,
        ):
            # resident transposed x2 shard. dma_start issue costs ~650 ns
            # each (serial on the Sync engine), so use few, big DMAs and
            # issue them in consumption order, interleaved with the first
            # panel's x1 tiles: the first matmul group is gated on DMA #1
            # (x2 j-block 0) + DMA #2 (x1 tile 0) only.
            x2t_t = res.tile([P, K_TILES, JS], BF16, tag="x2t")
            cmax = res.tile([P, M_TILES, JB], F32, tag="cmax")
            rmax = res.tile([P, M_TILES], F32, tag="rmax")

            def load_x2_chunk(jb, ks=slice(0, K_TILES)):
                js = slice(jb * JBLK, (jb + 1) * JBLK)
                nc.sync.dma_start(out=x2t_t[:, ks, js], in_=x2t[:, ks, js])

            def load_x1(m):
                a = x1pool.tile([P, K_TILES, P], BF16, tag="x1")
                nc.sync.dma_start(out=a[:], in_=x1t[m])
                return a

            # (m_start, m_count, order). Two 32-tile panels, both
            # j-block-outer: the first pass over 32 query tiles runs ~68 us
            # off x2 chunk 0 alone, which is ample time for chunks 1-3 to
            # land (an 8-tile first pass was measured to outrun the DMA and
            # trip a HAM re-throttle). j-block-outer also interleaves the
            # PSUM drains with matmuls, so nothing bunches after the last MM.
            parts = [(0, 32, "jb"), (32, 32, "jb")]

            def jb_outer(tiles, m0, cnt, skip=0):
                for jb in range(JB):
                    js = slice(jb * JBLK, (jb + 1) * JBLK)
                    for mi in range(cnt):
                        if jb == 0 and mi < skip:
                            continue
                        m = m0 + mi
                        ps = psum.tile([P, JBLK], F32, tag="ps")
                        for k in range(K_TILES):
                            nc.tensor.matmul(
                                ps[:], tiles[mi][:, k, :], x2t_t[:, k, js],
                                start=(k == 0), stop=(k == K_TILES - 1),
                            )
                        nc.vector.tensor_reduce(
                            cmax[:, m, jb : jb + 1], ps[:], axis=AX.X, op=ALU.max
                        )
                        if jb == JB - 1:
                            nc.vector.tensor_reduce(
                                rmax[:, m : m + 1], cmax[:, m, :], axis=AX.X, op=ALU.max
                            )
                            if (mi + 1) % 8 == 0:
                                nc.sync.dma_start(
                                    out=out[:, m - 7 : m + 1],
                                    in_=rmax[:, m - 7 : m + 1],
                                )

            def k_outer(tiles, m0, cnt, skip=0):
                for mi in range(cnt):
                    m = m0 + mi
                    pss = [psum.tile([P, JBLK], F32, tag="ps", name="ps") for _ in range(JB)]
                    for k in range(K_TILES):
                        for jb in range(JB):
                            js = slice(jb * JBLK, (jb + 1) * JBLK)
                            nc.tensor.matmul(
                                pss[jb][:], tiles[mi][:, k, :], x2t_t[:, k, js],
                                start=(k == 0), stop=(k == K_TILES - 1),
                            )
                    for jb in range(JB):
                        nc.vector.tensor_reduce(
                            cmax[:, m, jb : jb + 1], pss[jb][:], axis=AX.X, op=ALU.max
                        )
                    nc.vector.tensor_reduce(
                        rmax[:, m : m + 1], cmax[:, m, :], axis=AX.X, op=ALU.max
                    )

            # PE warm-up: matmuls on memset zeros, no DMA dependency.
            # They run during the initial DMA wait, flip the HAM clock gate
            # to 8/8, and finish about when the first real operands land --
            # so the real stream starts at full rate instead of paying the
            # ~3.4 us half-speed ramp.
            warm_a = res.tile([P, P], BF16, tag="warma")
            warm_b = res.tile([P, JBLK], BF16, tag="warmb")
            nc.any.memset(warm_a[:], 0)
            nc.any.memset(warm_b[:], 0)
            wps = psum.tile([P, JBLK], F32, tag="ps")
            for _ in range(16):
                nc.tensor.matmul(wps[:], warm_a[:], warm_b[:], start=True, stop=True)

            KH = K_TILES // 2
            for pi, (m0, cnt, order) in enumerate(parts):
                if pi == 0:
                    # first chunk as per-k slices (the proven v3 DMA shape):
                    # the opening groups are gated on k 0..3 + one x1 tile
                    for k in range(KH):
                        nc.sync.dma_start(
                            out=x2t_t[:, k, 0:JBLK], in_=x2t[:, k, 0:JBLK]
                        )
                    tiles = [load_x1(m0 + mi) for mi in range(4)]
                    for k in range(KH, K_TILES):
                        nc.sync.dma_start(
                            out=x2t_t[:, k, 0:JBLK], in_=x2t[:, k, 0:JBLK]
                        )
                    tiles += [load_x1(m0 + mi) for mi in range(4, cnt)]
                    # m0..m3, j-block 0: accumulate k 0..3 while the second
                    # k-half of the chunk is still in flight
                    open_ps = []
                    for mi in range(4):
                        ps = psum.tile([P, JBLK], F32, tag="ps")
                        for k in range(KH):
                            nc.tensor.matmul(
                                ps[:], tiles[mi][:, k, :], x2t_t[:, k, 0:JBLK],
                                start=(k == 0), stop=False,
                            )
                        open_ps.append(ps)
                    for mi in range(4):
                        ps = open_ps[mi]
                        for k in range(KH, K_TILES):
                            nc.tensor.matmul(
                                ps[:], tiles[mi][:, k, :], x2t_t[:, k, 0:JBLK],
                                start=False, stop=(k == K_TILES - 1),
                            )
                        nc.vector.tensor_reduce(
                            cmax[:, m0 + mi, 0:1], ps[:], axis=AX.X, op=ALU.max
                        )
                    for jb in range(1, JB):
                        load_x2_chunk(jb)
                else:
                    tiles = [load_x1(m0 + mi) for mi in range(cnt)]
                (jb_outer if order == "jb" else k_outer)(
                    tiles, m0, cnt, skip=4 if pi == 0 else 0
                )

    nc.finalize()
    return nc


_cache = {}


def _get_nc():
    if "nc" not in _cache:
        _cache["nc"] = build_nc()
    return _cache["nc"]


def _prep_inputs(x1, x2):
    """Host-side prep: row-normalize, TF32-round, transpose + tile + shard."""
    x1 = np.ascontiguousarray(x1, dtype=np.float32)
    x2 = np.ascontiguousarray(x2, dtype=np.float32)
    eps = np.float32(1e-8)
    n1 = np.maximum(np.sqrt(np.einsum("ij,ij->i", x1, x1)), eps)
    n2 = np.maximum(np.sqrt(np.einsum("ij,ij->i", x2, x2)), eps)
    x1 = (x1 / n1[:, None]).astype(ml_dtypes.bfloat16)
    x2 = (x2 / n2[:, None]).astype(ml_dtypes.bfloat16)

    # [N1, D] -> [m, dp, k, q] with x1t[m, dp, k, q] = x1[m*128+q, k*128+dp]
    x1t = np.ascontiguousarray(
        x1.reshape(M_TILES, P, K_TILES, P).transpose(0, 3, 2, 1)
    )

    in_maps = []
    for c in range(NCORES):
        sl = slice(c * JS, (c + 1) * JS)
        # [JS, D] -> [dp, k, j] with x2t[dp, k, j] = x2[sl][j, k*128+dp]
        x2t = np.ascontiguousarray(
            x2[sl].T.reshape(K_TILES, P, JS).transpose(1, 0, 2)
        )
        in_maps.append({"x1t": x1t, "x2t": x2t})
    return in_maps


def run(x1, x2, trace=False):
    nc = _get_nc()
    in_maps = _prep_inputs(x1, x2)
    res = run_bass_kernel_spmd(nc, in_maps, core_ids=list(range(NCORES)), trace=trace)
    # device output is [q_within_tile, m_tile]; out[m*128+q] = arr[q, m]
    parts = [np.asarray(res.results[c]["out"]).reshape(P, M_TILES) for c in range(NCORES)]
    out = np.maximum.reduce(parts).T.ravel().astype(np.float32)
    return np.ascontiguousarray(out), res


def kernel(x1, x2):
    out, _ = run(np.asarray(x1), np.asarray(x2), trace=False)
    return out


# revision 16
# speedup vs baseline: 1.4906x; 1.0003x over previous
"""MaxSimilarity (cosine-sim row-max) Trainium2 kernel.

out[i] = max_j  (x1[i] . x2[j]) / max(||x1[i]|| * ||x2[j]||, 1e-8)
x1: [8192, 1024] f32, x2: [16384, 1024] f32, out: [8192] f32.

Strategy (8 NeuronCores):
- Shard x2 rows 8-way (2048 rows/core); replicate x1. Each core computes the
  row-max over its j-shard for all 8192 queries; host combines shards with
  elementwise max.
- Rows of x1 and x2 are normalized to unit length on the host, so the device
  kernel is a pure matmul + row-max. Matmul runs in bf16: a single term
  gives ~1.3e-3 relative error on this data, far inside the 2e-2 gate.
  bf16 streams at 1 cycle/row like float32r, but (unlike float32r, whose
  4-byte weight path forces a full weight reload with every matmul) it gets
  Fast Weight Load, which shaves the per-matmul LDWEIGHTS overhead; it also
  halves the HBM traffic. 2048 matmuls x 512 cycles per core.
- Loop structure is j-block-outer over two resident panels of 32 query
  tiles: each pass over a panel needs only one 2 MB j-chunk of x2, so the
  PE starts after ~2.5 MB of DMA instead of waiting for the full 8 MB x2
  shard (which cost 35 us of dead PE time m-outer), and the ~68 us first
  pass gives the remaining chunks ample time to land (shorter first passes
  were measured to outrun the DMA and trip a HAM re-throttle).
- PSUM tiles [128 q, 512 j] are drained on DVE with a reduce-max over j into
  a per-(m,jb) column; after a panel's last pass each query tile's 4 block
  maxima are reduced and the result is written out once, contiguously, in
  [q_within_tile, m_tile] layout (the host untransposes -- a direct
  (m p)-ordered DMA scatters 8192 4-byte writes to HBM and costs ~25 us in
  write-completion latency).
"""

import ml_dtypes
import numpy as np

import concourse.bacc as bacc
import concourse.mybir as mybir
import concourse.tile as tile
from concourse.bass_utils import run_bass_kernel_spmd

N1, N2, D = 8192, 16384, 1024
P = 128
NCORES = 8
JS = N2 // NCORES          # 2048 j per core
JBLK = 512                 # psum moving free dim (one bank of fp32)
JB = JS // JBLK            # 4 psum blocks per core
M_TILES = N1 // P          # 64
K_TILES = D // P           # 8
MP = 32                    # m-tiles per panel (16 MB of x1 resident)
PARTS = M_TILES // MP      # 2 panels

F32 = mybir.dt.float32
BF16 = mybir.dt.bfloat16
ALU = mybir.AluOpType
AX = mybir.AxisListType


def build_nc():
    nc = bacc.Bacc(trn_type="TRN2")

    x1t = nc.dram_tensor("x1t", [M_TILES, P, K_TILES, P], BF16, kind="ExternalInput")
    x2t = nc.dram_tensor("x2t", [P, K_TILES, JS], BF16, kind="ExternalInput")
    out = nc.dram_tensor("out", [P, M_TILES], F32, kind="ExternalOutput")

    with tile.TileContext(nc) as tc:
        with (
            tc.tile_pool(name="resident", bufs=1) as res,
            tc.tile_pool(name="x1pool", bufs=MP) as x1pool,
            tc.tile_pool(name="psum", bufs=8, space="PSUM") as psum,
        ):
            # resident transposed x2 shard. dma_start issue costs ~650 ns
            # each (serial on the Sync engine), so use few, big DMAs and
            # issue them in consumption order, interleaved with the first
            # panel's x1 tiles: the first matmul group is gated on DMA #1
            # (x2 j-block 0) + DMA #2 (x1 tile 0) only.
            x2t_t = res.tile([P, K_TILES, JS], BF16, tag="x2t")
            cmax = res.tile([P, M_TILES, JB], F32, tag="cmax")
            rmax = res.tile([P, M_TILES], F32, tag="rmax")

            def load_x2_chunk(jb, ks=slice(0, K_TILES)):
                js = slice(jb * JBLK, (jb + 1) * JBLK)
                nc.sync.dma_start(out=x2t_t[:, ks, js], in_=x2t[:, ks, js])

            def load_x1(m):
                a = x1pool.tile([P, K_TILES, P], BF16, tag="x1")
                nc.sync.dma_start(out=a[:], in_=x1t[m])
                return a

            # (m_start, m_count, order). Two 32-tile panels, both
            # j-block-outer: the first pass over 32 query tiles runs ~68 us
            # off x2 chunk 0 alone, which is ample time for chunks 1-3 to
            # land (an 8-tile first pass was measured to outrun the DMA and
            # trip a HAM re-throttle). j-block-outer also interleaves the
            # PSUM drains with matmuls, so nothing bunches after the last MM.
            parts = [(0, 32, "jb"), (32, 32, "jb")]

            def jb_outer(tiles, m0, cnt, skip=0):
                for jb in range(JB):
                    js = slice(jb * JBLK, (jb + 1) * JBLK)
                    for mi in range(cnt):
                        if jb == 0 and mi < skip:
                            continue
                        m = m0 + mi
                        ps = psum.tile([P, JBLK], F32, tag="ps")
                        for k in range(K_TILES):
                            nc.tensor.matmul(
                                ps[:], tiles[mi][:, k, :], x2t_t[:, k, js],
                                start=(k == 0), stop=(k == K_TILES - 1),
                            )
                        nc.vector.tensor_reduce(
                            cmax[:, m, jb : jb + 1], ps[:], axis=AX.X, op=ALU.max
                        )
                        if jb == JB - 1:
                            nc.vector.tensor_reduce(
                                rmax[:, m : m + 1], cmax[:, m, :], axis=AX.X, op=ALU.max
                            )
                            if (mi + 1) % 8 == 0:
                                nc.sync.dma_start(
                                    out=out[:, m - 7 : m + 1],
                                    in_=rmax[:, m - 7 : m + 1],
                                )

            def k_outer(tiles, m0, cnt, skip=0):
                for mi in range(cnt):
                    m = m0 + mi
                    pss = [psum.tile([P, JBLK], F32, tag="ps", name="ps") for _ in range(JB)]
                    for k in range(K_TILES):
                        for jb in range(JB):
                            js = slice(jb * JBLK, (jb + 1) * JBLK)
                            nc.tensor.matmul(
                                pss[jb][:], tiles[mi][:, k, :], x2t_t[:, k, js],
                                start=(k == 0), stop=(k == K_TILES - 1),
                            )
                    for jb in range(JB):
                        nc.vector.tensor_reduce(
                            cmax[:, m, jb : jb + 1], pss[jb][:], axis=AX.X, op=ALU.max
                        )
                    nc.vector.tensor_reduce(
                        rmax[:, m : m + 1], cmax[:, m, :], axis=AX.X, op=ALU.max
                    )

            # PE warm-up: matmuls on memset zeros, no DMA dependency.
            # They run during the initial DMA wait, flip the HAM clock gate
            # to 8/8, and finish about when the first real operands land --
            # so the real stream starts at full rate instead of paying the
            # ~3.4 us half-speed ramp.
            warm_a = res.tile([P, P], BF16, tag="warma")
            warm_b = res.tile([P, JBLK], BF16, tag="warmb")
            nc.any.memset(warm_a[:], 0)
            nc.any.memset(warm_b[:], 0)
            wps = psum.tile([P, JBLK], F32, tag="ps")
            for _ in range(12):
                nc.tensor.matmul(wps[:], warm_a[:], warm_b[:], start=True, stop=True)

            KH = K_TILES // 2
            for pi, (m0, cnt, order) in enumerate(parts):
                if pi == 0:
                    # x1 tile 0 first, then the k 0..3 slices of chunk 0
                    # (the proven v3 DMA shape) -- together they gate the
                    # opening groups, so issue exactly them before anything
                    tiles = [load_x1(m0)]
                    for k in range(KH):
                        nc.sync.dma_start(
                            out=x2t_t[:, k, 0:JBLK], in_=x2t[:, k, 0:JBLK]
                        )
                    tiles += [load_x1(m0 + mi) for mi in range(1, 4)]
                    for k in range(KH, K_TILES):
                        nc.sync.dma_start(
                            out=x2t_t[:, k, 0:JBLK], in_=x2t[:, k, 0:JBLK]
                        )
                    tiles += [load_x1(m0 + mi) for mi in range(4, cnt)]
                    # m0..m3, j-block 0: accumulate k 0..3 while the second
                    # k-half of the chunk is still in flight
                    open_ps = []
                    for mi in range(4):
                        ps = psum.tile([P, JBLK], F32, tag="ps")
                        for k in range(KH):
                            nc.tensor.matmul(
                                ps[:], tiles[mi][:, k, :], x2t_t[:, k, 0:JBLK],
                                start=(k == 0), stop=False,
                            )
                        open_ps.append(ps)
                    for mi in range(4):
                        ps = open_ps[mi]
                        for k in range(KH, K_TILES):
                            nc.tensor.matmul(
                                ps[:], tiles[mi][:, k, :], x2t_t[:, k, 0:JBLK],
                                start=False, stop=(k == K_TILES - 1),
                            )
                        nc.vector.tensor_reduce(
                            cmax[:, m0 + mi, 0:1], ps[:], axis=AX.X, op=ALU.max
                        )
                    for jb in range(1, JB):
                        load_x2_chunk(jb)
                else:
                    tiles = [load_x1(m0 + mi) for mi in range(cnt)]
                (jb_outer if order == "jb" else k_outer)(
                    tiles, m0, cnt, skip=4 if pi == 0 else 0
                )

    nc.finalize()
    return nc


_cache = {}


def _get_nc():
    if "nc" not in _cache:
        _cache["nc"] = build_nc()
    return _cache["nc"]


def _prep_inputs(x1, x2):
    """Host-side prep: row-normalize, TF32-round, transpose + tile + shard."""
    x1 = np.ascontiguousarray(x1, dtype=np.float32)
    x2 = np.ascontiguousarray(x2, dtype=np.float32)
    eps = np.float32(1e-8)
    n1 = np.maximum(np.sqrt(np.einsum("ij,ij->i", x1, x1)), eps)
    n2 = np.maximum(np.sqrt(np.einsum("ij,ij->i", x2, x2)), eps)
    x1 = (x1 / n1[:, None]).astype(ml_dtypes.bfloat16)
    x2 = (x2 / n2[:, None]).astype(ml_dtypes.bfloat16)

    # [N1, D] -> [m, dp, k, q] with x1t[m, dp, k, q] = x1[m*128+q, k*128+dp]
    x1t = np.ascontiguousarray(
        x1.reshape(M_TILES, P, K_TILES, P).transpose(0, 3, 2, 1)
    )

    in_maps = []
    for c in range(NCORES):
        sl = slice(c * JS, (c + 1) * JS)
        # [JS, D] -> [dp, k, j] with x2t[dp, k, j] = x2[sl][j, k*128+dp]
        x2t = np.ascontiguousarray(
            x2[sl].T.reshape(K_TILES, P, JS).transpose(1, 0, 2)
        )
        in_maps.append({"x1t": x1t, "x2t": x2t})
    return in_maps


def run(x1, x2, trace=False):
    nc = _get_nc()
    in_maps = _prep_inputs(x1, x2)
    res = run_bass_kernel_spmd(nc, in_maps, core_ids=list(range(NCORES)), trace=trace)
    # device output is [q_within_tile, m_tile]; out[m*128+q] = arr[q, m]
    parts = [np.asarray(res.results[c]["out"]).reshape(P, M_TILES) for c in range(NCORES)]
    out = np.maximum.reduce(parts).T.ravel().astype(np.float32)
    return np.ascontiguousarray(out), res


def kernel(x1, x2):
    out, _ = run(np.asarray(x1), np.asarray(x2), trace=False)
    return out
